# revision 1
# baseline (speedup 1.0000x reference)
"""
Trainium2 Bass kernel for nn_DenseFeatureNumericEmbedding.

Computes, per feature f (F=128 independent tiny MLPs):
    h[b,f,:]   = relu(x[b,f] * w1[f,:] + b1[f,:])            # [B, F, H]
    out[b,f,:] = h[b,f,:] @ w2[f,:,:] + b2[f,:]              # [B, F, E]
    returns out.reshape(B, F*E)                              # [16384, 4096] fp32

Sharding: data-parallel over batch across 8 NeuronCores (2048 rows/core),
params replicated. No collectives; host concatenates the 8 output shards.

Per-core dataflow (per 512-batch chunk, per quad of 4 features):
  L1   TensorE: K=2 matmuls, stationary [w1[f]; b1[f]], moving [xT[f]; ones]
       -> preactT [H=128, 512] in PSUM (bias folded into the matmul).
  RELU ScalarE activation(Relu) / VectorE tensor_scalar_max(0) split,
       PSUM -> SBUF, cast to bf16 -> hT [128, 2048].
  L2   TensorE: per feature, stationary w2[f] [H,E], moving hT -> col-tiled
       4 features into one PSUM bank -> outT [FE=128, 512].
  B2+COPY ScalarE activation(Identity, bias=b2 column) PSUM -> SBUF.
  TRANS TensorE transpose -> PSUM [b, fe], VectorE copy -> SBUF staging.
  DMA  store with 512B+ contiguous runs in DRAM (optionally bf16 staging
       with SWDGE dtype-cast DMA to fp32).
"""

import sys

sys.path.insert(0, "/opt/trn_rl_repo")

import numpy as np
import ml_dtypes

import concourse.bass as bass
import concourse.tile as tile
from concourse import bacc, mybir
from concourse.bass_utils import run_bass_kernel_spmd

BF16 = ml_dtypes.bfloat16

B = 16384
F = 128
H = 128
E = 32
NCORES = 8
BL = B // NCORES          # 2048 rows per core
CHUNK = 512               # batch columns per inner tile (1 PSUM bank fp32)
NCHUNK = BL // CHUNK      # 4
NQUAD = F // 4            # 32 quads of 4 features

CONFIG = {
    "ACT_OF_8": 6,     # of every 8 relu instrs, this many on ScalarE
    "OUT_BF16": False,  # bf16 out-path + SWDGE cast-DMA to fp32
    "NO_PE_TR": False,  # DVE 32x32 block transpose instead of PE transpose
    "L1_F32R": False,   # run L1 matmuls in float32r instead of bf16
    "LDWOPT": False,    # pass --enable-ldw-opt=true to walrus
    "VARIANT_ID": 0,    # busts the NEFF cache between variants
}

_COMPILED = None
_ORIG_RUN_COMMAND = None


def _install_ldwopt_patch():
    import concourse.bass_utils as bu
    global _ORIG_RUN_COMMAND
    if _ORIG_RUN_COMMAND is None:
        _ORIG_RUN_COMMAND = bu.run_command

    def patched(cmd, *a, **kw):
        if CONFIG["LDWOPT"] and isinstance(cmd, list):
            cmd = ["--enable-ldw-opt=true" if c == "--enable-ldw-opt=false"
                   else c for c in cmd]
        return _ORIG_RUN_COMMAND(cmd, *a, **kw)

    bu.run_command = patched


def _build_bass():
    _install_ldwopt_patch()
    nc = bacc.Bacc("TRN2", target_bir_lowering=False, debug=False,
                   num_devices=NCORES)
    dt = mybir.dt
    out_bf16 = CONFIG["OUT_BF16"]
    no_pe_tr = CONFIG["NO_PE_TR"]
    assert not no_pe_tr or out_bf16, "NO_PE_TR requires OUT_BF16"
    l1_f32r = CONFIG["L1_F32R"]
    l1_dt = dt.float32r if l1_f32r else dt.bfloat16
    o_dt = dt.bfloat16 if out_bf16 else dt.float32
    act_of_8 = CONFIG["ACT_OF_8"]

    xt2 = nc.dram_tensor("xt2", [2 * F, BL], l1_dt, kind="ExternalInput").ap()
    w1b1q = nc.dram_tensor("w1b1q", [128, F * H], l1_dt, kind="ExternalInput").ap()
    w2s = nc.dram_tensor("w2s", [H, F * E], dt.bfloat16, kind="ExternalInput").ap()
    b2qs = nc.dram_tensor("b2qs", [128, NQUAD], dt.float32, kind="ExternalInput").ap()
    eye = nc.dram_tensor("eye", [128, 128], o_dt, kind="ExternalInput").ap()
    out = nc.dram_tensor("out", [BL, F * E], dt.float32, kind="ExternalOutput").ap()

    # DRAM views
    # xt2 rows: 8q + 2j + r  (q quad, j feature-in-quad, r 0=x / 1=ones)
    xt2_r = xt2.rearrange("(q g) n -> g q n", g=8)       # [8, NQUAD, BL]
    # out rows: 512c + 128jj + p
    out_r = out.rearrange("(c jj p) n -> c p jj n", jj=4, p=128)  # [NCHUNK,128,4,FE]
    if CONFIG["NO_PE_TR"]:
        # bf16 scratch holding outT (transposed output), [FE, BL]
        scr = nc.dram_tensor("outT_scr", [F * E, BL], dt.bfloat16).ap()
        # rows (q2, s, p): fe = 256*q2 + 128*s + p
        scr_r = scr.rearrange("(q2 s p) n -> q2 p s n", s=2, p=128)

    for _ in range(CONFIG["VARIANT_ID"]):
        nc.sync.nop()

    with tile.TileContext(nc) as tc:
        with (
            tc.tile_pool(name="params", bufs=1) as params,
            tc.tile_pool(name="xq", bufs=2) as xq_pool,
            tc.tile_pool(name="h", bufs=4) as h_pool,
            tc.tile_pool(name="outT", bufs=4) as outT_pool,
            tc.tile_pool(name="stage", bufs=2) as stage_pool,
            tc.tile_pool(name="outq", bufs=4) as outq_pool,
            tc.tile_pool(name="pre", bufs=2, space="PSUM") as pre_pool,
            tc.tile_pool(name="pout", bufs=2, space="PSUM") as pout_pool,
            tc.tile_pool(name="ptr", bufs=2, space="PSUM") as ptr_pool,
        ):
            w1b1q_sb = params.tile([128, F * H], l1_dt, tag="w1b1q")
            nc.sync.dma_start(out=w1b1q_sb[:], in_=w1b1q[:])
            w2_sb = params.tile([H, F * E], dt.bfloat16, tag="w2s")
            nc.sync.dma_start(out=w2_sb[:], in_=w2s[:])
            b2_sb = params.tile([128, NQUAD], dt.float32, tag="b2qs")
            nc.sync.dma_start(out=b2_sb[:], in_=b2qs[:])
            eye_sb = params.tile([128, 128], o_dt, tag="eye")
            nc.sync.dma_start(out=eye_sb[:], in_=eye[:])

            relu_idx = 0
            for c in range(NCHUNK):
                # xq[32j + r, 512q + cc] = xt2[8q + 2j + r, 512c + cc]
                xq = xq_pool.tile([128, NQUAD * CHUNK], l1_dt, tag="xq")
                for j in range(4):
                    nc.sync.dma_start(
                        out=xq[32 * j:32 * j + 2, :].rearrange(
                            "r (q n) -> r q n", n=CHUNK),
                        in_=xt2_r[2 * j:2 * j + 2, :, bass.ts(c, CHUNK)],
                    )
                if out_bf16 and not no_pe_tr:
                    stage = stage_pool.tile([128, 4, F * E], dt.bfloat16,
                                            tag="stage")
                scr_dmas = []

                for q in range(NQUAD):
                    # ---- L1: 4 features, row-groups 0..3, K=2 matmuls ----
                    if no_pe_tr and q % 2 == 0:
                        pout2 = pout_pool.tile([128, 2 * CHUNK], dt.float32,
                                               tag="pout2")
                        outT2 = outT_pool.tile([128, 2 * CHUNK], dt.bfloat16,
                                               tag="outT2")
                    pre_a = pre_pool.tile([128, 2 * CHUNK], dt.float32, tag="pre")
                    pre_b = pre_pool.tile([128, 2 * CHUNK], dt.float32, tag="pre")
                    for j in range(4):
                        tgt = pre_a if j < 2 else pre_b
                        nc.tensor.matmul(
                            tgt[:, bass.ts(j % 2, CHUNK)],
                            lhsT=w1b1q_sb[32 * j:32 * j + 2, bass.ts(q, H)],
                            rhs=xq[32 * j:32 * j + 2, bass.ts(q, CHUNK)],
                            start=True, stop=True,
                            tile_position=(32 * j, 0),
                        )

                    # ---- relu + cast bf16, split ACT / DVE ----
                    hT = h_pool.tile([128, 4 * CHUNK], dt.bfloat16, tag="h")
                    for half, hsrc in ((0, pre_a), (1, pre_b)):
                        dst = hT[:, bass.ts(half, 2 * CHUNK)]
                        if relu_idx % 8 < act_of_8:
                            nc.scalar.activation(
                                dst, hsrc[:], mybir.ActivationFunctionType.Relu)
                        else:
                            nc.vector.tensor_scalar_max(dst, hsrc[:], 0.0)
                        relu_idx += 1

                    # ---- L2: 4 features col-tiled into one PSUM bank ----
                    if no_pe_tr:
                        pout = pout2[:, bass.ts(q % 2, CHUNK)]
                    else:
                        pout = pout_pool.tile([128, CHUNK], dt.float32,
                                              tag="pout")
                    for j in range(4):
                        f = 4 * q + j
                        nc.tensor.matmul(
                            pout[32 * j:32 * j + 32, :],
                            lhsT=w2_sb[:, bass.ts(f, E)],
                            rhs=hT[:, bass.ts(j, CHUNK)],
                            start=True, stop=True,
                            tile_position=(0, 32 * j),
                        )

                    # ---- + b2, PSUM -> SBUF ----
                    if no_pe_tr:
                        # bias-add + cast on VectorE; ScalarE is the busier
                        outT = outT2[:, bass.ts(q % 2, CHUNK)]
                        nc.vector.tensor_scalar_add(
                            outT, pout[:], b2_sb[:, q:q + 1])
                        if q % 2 == 1:
                            # outT straight to DRAM scratch (2 quads batched)
                            scr_dmas.append(nc.sync.dma_start(
                                out=scr_r[q // 2, :, :, bass.ts(c, CHUNK)],
                                in_=outT2[:].rearrange("p (s n) -> p s n",
                                                       n=CHUNK)))
                        continue
                    outT = outT_pool.tile([128, CHUNK], o_dt, tag="outT")
                    nc.scalar.activation(
                        outT[:], pout[:],
                        mybir.ActivationFunctionType.Identity,
                        bias=b2_sb[:, q:q + 1],
                    )

                    # ---- transpose [fe, b] -> [b, fe] via TensorE ----
                    ptr = ptr_pool.tile([128, CHUNK], o_dt, tag="ptr")
                    for jj in range(4):
                        nc.tensor.transpose(
                            ptr[:, bass.ts(jj, 128)],
                            outT[:, bass.ts(jj, 128)],
                            eye_sb[:],
                        )

                    if out_bf16:
                        nc.vector.tensor_copy(
                            stage[:, :, bass.ts(q, 128)], ptr[:])
                    else:
                        outq = outq_pool.tile([128, CHUNK], dt.float32,
                                              tag="outq")
                        nc.vector.tensor_copy(outq[:], ptr[:])
                        # rows 512c+128jj+p, cols 128q..128q+128
                        nc.sync.dma_start(
                            out=out_r[c, :, :, bass.ts(q, 128)],
                            in_=outq[:].rearrange("p (jj n) -> p jj n", n=128),
                        )

                if out_bf16 and no_pe_tr:
                    from concourse.tile import add_dep_helper
                    for bsub in range(4):
                        xp = stage_pool.tile([128, F * E], dt.bfloat16,
                                             tag="xp")
                        tr = nc.sync.dma_start(
                            out=xp[:],
                            in_=scr[:, 512 * c + 128 * bsub:
                                    512 * c + 128 * bsub + 128],
                            transpose=True)
                        for sd in scr_dmas:
                            add_dep_helper(tr.ins, sd.ins,
                                           reason="xbar reads chunk scratch")
                        # cast bf16 -> fp32, contiguous 16KB DRAM rows
                        nc.gpsimd.dma_start(
                            out=out[512 * c + 128 * bsub:
                                    512 * c + 128 * bsub + 128, :],
                            in_=xp[:])
                elif out_bf16:
                    nc.gpsimd.dma_start(out=out_r[c], in_=stage[:])

    nc.compile()
    return nc


def _prep_inputs(x, w1, b1, w2, b2):
    """Host-side packing of parameters + per-core x shards."""
    l1_np = np.float32 if CONFIG["L1_F32R"] else BF16
    o_np = BF16 if CONFIG["OUT_BF16"] else np.float32

    w1b1q = np.zeros((128, F * H), dtype=l1_np)
    for f in range(F):
        q, j = divmod(f, 4)
        w1b1q[32 * j + 0, H * q:H * q + H] = w1[f].astype(l1_np)
        w1b1q[32 * j + 1, H * q:H * q + H] = b1[f].astype(l1_np)

    w2s = np.ascontiguousarray(
        w2.transpose(1, 0, 2).reshape(H, F * E)).astype(BF16)
    # b2qs[32j + e, q] = b2[4q + j, e]
    b2qs = np.ascontiguousarray(
        b2.reshape(NQUAD, 4, E).transpose(1, 2, 0).reshape(128, NQUAD)
    ).astype(np.float32)
    eye = np.eye(128, dtype=o_np)

    in_maps = []
    for core in range(NCORES):
        xs = x[core * BL:(core + 1) * BL]          # [BL, F]
        xt2 = np.empty((2 * F, BL), dtype=l1_np)
        xt2[0::2] = xs.T.astype(l1_np)
        xt2[1::2] = l1_np(1.0)
        in_maps.append({
            "xt2": xt2, "w1b1q": w1b1q, "w2s": w2s,
            "b2qs": b2qs, "eye": eye,
        })
    return in_maps


def _get_compiled():
    global _COMPILED
    if _COMPILED is None:
        _COMPILED = _build_bass()
    return _COMPILED


def reset_compiled():
    global _COMPILED
    _COMPILED = None


def kernel(x, w1, b1, w2, b2, _trace=False, _trace_kwargs=None):
    nc = _get_compiled()
    in_maps = _prep_inputs(
        np.asarray(x, dtype=np.float32), np.asarray(w1, dtype=np.float32),
        np.asarray(b1, dtype=np.float32), np.asarray(w2, dtype=np.float32),
        np.asarray(b2, dtype=np.float32))
    res = run_bass_kernel_spmd(
        nc, in_maps, core_ids=list(range(NCORES)),
        trace=_trace, **(_trace_kwargs or {}))
    shards = [np.asarray(res.results[i]["out"]) for i in range(NCORES)]
    full = np.concatenate(shards, axis=0).astype(np.float32)
    if _trace:
        return full, res
    return full


if __name__ == "__main__":
    rng = np.random.default_rng(0)
    x = rng.standard_normal((B, F), dtype=np.float32)
    w1 = rng.standard_normal((F, H), dtype=np.float32)
    b1 = rng.standard_normal((F, H), dtype=np.float32)
    w2 = (rng.standard_normal((F, H, E), dtype=np.float32) / np.sqrt(H)).astype(np.float32)
    b2 = rng.standard_normal((F, E), dtype=np.float32) / np.sqrt(H)
    got = kernel(x=x, w1=w1, b1=b1, w2=w2, b2=b2)
    h = np.maximum(x[:, :, None] * w1[None] + b1[None], 0.0)
    want = (np.einsum("bfh,fhe->bfe", h, w2) + b2[None]).reshape(B, F * E)
    err = np.abs(got - want).max() / np.abs(want).max()
    print("self-test scale-relative max err:", err)



# revision 2
# speedup vs baseline: 1.6298x; 1.6298x over previous
"""
Trainium2 Bass kernel for nn_DenseFeatureNumericEmbedding.

Computes, per feature f (F=128 independent tiny MLPs):
    h[b,f,:]   = relu(x[b,f] * w1[f,:] + b1[f,:])            # [B, F, H]
    out[b,f,:] = h[b,f,:] @ w2[f,:,:] + b2[f,:]              # [B, F, E]
    returns out.reshape(B, F*E)                              # [16384, 4096] fp32

Sharding: data-parallel over batch across 8 NeuronCores (2048 rows/core),
params replicated. No collectives; host concatenates the 8 output shards.

v2 dataflow (vs v1): NO on-device transpose. The kernel writes the output
transposed (outT [F*E, BL] bf16) and the host transposes + casts to fp32.
This removes 512 PE transposes (132us) and 128 DVE copies (86us) per core.

Per-core, per 512-batch chunk, per quad of 4 features (2 pairs):
  L1   TensorE: per feature, K=2 matmul, stationary [w1[f]; b1[f]],
       moving [xT[f]; ones] -> pre [H=128, 512] fp32 in PSUM (bias folded).
       Pair granularity: 2 features -> one [128, 1024] PSUM tile (2 banks).
  RELU ScalarE activation(Relu) / VectorE tensor_scalar_max(0) split,
       PSUM -> SBUF, cast to bf16 -> h pair tile [128, 1024].
  L2   TensorE: per feature, stationary w2[f] [H,E=32], moving h ->
       col-tiled 4 features into one PSUM bank -> outT [FE=128, 512].
  COPY +b2 fused: ScalarE activation(Identity, bias) or VectorE
       tensor_scalar_add, PSUM -> SBUF bf16.
  DMA  outT tile [128, 512] -> DRAM outT[128q:128q+128, 512c:512c+512]
       (1KB contiguous per partition row).

PSUM budget: pre pool 3 bufs x 2 banks + pout 2 bufs x 1 bank = 8/8 banks.
"""

import sys

sys.path.insert(0, "/opt/trn_rl_repo")

import numpy as np
import ml_dtypes

import concourse.bass as bass
import concourse.tile as tile
from concourse import bacc, mybir
from concourse.bass_utils import run_bass_kernel_spmd

BF16 = ml_dtypes.bfloat16

B = 16384
F = 128
H = 128
E = 32
NCORES = 8
BL = B // NCORES          # 2048 rows per core
CHUNK = 512               # batch columns per inner tile (1 PSUM bank fp32)
NCHUNK = BL // CHUNK      # 4
NQUAD = F // 4            # 32 quads of 4 features

CONFIG = {
    # engine split patterns: 'A' = ScalarE (Act), 'D' = VectorE (DVE)
    "RELU_PAT": "ADAD",    # per-pair relu instrs, cycled
    "COPY_PAT": "AAAD",    # per-quad copy+bias instrs, cycled
    "PRE_BUFS": 3,         # PSUM bufs for pre tiles (2 banks each)
    "POUT_BUFS": 2,        # PSUM bufs for L2 out (1 bank each)
    "VARIANT_ID": 10,      # busts the NEFF cache between variants
}

_COMPILED = None


def _build_bass():
    nc = bacc.Bacc("TRN2", target_bir_lowering=False, debug=False,
                   num_devices=NCORES)
    dt = mybir.dt

    xt2 = nc.dram_tensor("xt2", [2 * F, BL], dt.bfloat16, kind="ExternalInput").ap()
    w1b1q = nc.dram_tensor("w1b1q", [128, F * H], dt.bfloat16, kind="ExternalInput").ap()
    w2s = nc.dram_tensor("w2s", [H, F * E], dt.bfloat16, kind="ExternalInput").ap()
    b2qs = nc.dram_tensor("b2qs", [128, NQUAD], dt.float32, kind="ExternalInput").ap()
    out = nc.dram_tensor("out", [F * E, BL], dt.bfloat16, kind="ExternalOutput").ap()

    # DRAM view of xt2: rows 8q + 2j + r (q quad, j feature-in-quad, r 0=x/1=ones)
    xt2_r = xt2.rearrange("(q g) n -> g q n", g=8)       # [8, NQUAD, BL]

    for _ in range(CONFIG["VARIANT_ID"]):
        nc.sync.nop()

    relu_pat = CONFIG["RELU_PAT"]
    copy_pat = CONFIG["COPY_PAT"]

    with tile.TileContext(nc) as tc:
        with (
            tc.tile_pool(name="params", bufs=1) as params,
            tc.tile_pool(name="xq", bufs=2) as xq_pool,
            tc.tile_pool(name="h", bufs=6) as h_pool,
            tc.tile_pool(name="outq", bufs=4) as outq_pool,
            tc.tile_pool(name="pre", bufs=CONFIG["PRE_BUFS"], space="PSUM") as pre_pool,
            tc.tile_pool(name="pout", bufs=CONFIG["POUT_BUFS"], space="PSUM") as pout_pool,
        ):
            w1b1q_sb = params.tile([128, F * H], dt.bfloat16, tag="w1b1q")
            nc.sync.dma_start(out=w1b1q_sb[:], in_=w1b1q[:])
            w2_sb = params.tile([H, F * E], dt.bfloat16, tag="w2s")
            nc.sync.dma_start(out=w2_sb[:], in_=w2s[:])
            b2_sb = params.tile([128, NQUAD], dt.float32, tag="b2qs")
            nc.sync.dma_start(out=b2_sb[:], in_=b2qs[:])

            relu_idx = 0
            copy_idx = 0
            for c in range(NCHUNK):
                # xq[32j + r, 512q + cc] = xt2[8q + 2j + r, 512c + cc]
                xq = xq_pool.tile([128, NQUAD * CHUNK], dt.bfloat16, tag="xq")
                for j in range(4):
                    nc.sync.dma_start(
                        out=xq[32 * j:32 * j + 2, :].rearrange(
                            "r (q n) -> r q n", n=CHUNK),
                        in_=xt2_r[2 * j:2 * j + 2, :, bass.ts(c, CHUNK)],
                    )

                for q in range(NQUAD):
                    hpair = []
                    for p in range(2):          # pair p: features 4q+2p, 4q+2p+1
                        pre = pre_pool.tile([128, 2 * CHUNK], dt.float32,
                                            tag="pre")
                        for jj in range(2):
                            j = 2 * p + jj
                            nc.tensor.matmul(
                                pre[:, bass.ts(jj, CHUNK)],
                                lhsT=w1b1q_sb[32 * j:32 * j + 2, bass.ts(q, H)],
                                rhs=xq[32 * j:32 * j + 2, bass.ts(q, CHUNK)],
                                start=True, stop=True,
                                tile_position=(32 * j, 0),
                            )
                        hT = h_pool.tile([128, 2 * CHUNK], dt.bfloat16, tag="h")
                        if relu_pat[relu_idx % len(relu_pat)] == "A":
                            nc.scalar.activation(
                                hT[:], pre[:], mybir.ActivationFunctionType.Relu)
                        else:
                            nc.vector.tensor_scalar_max(hT[:], pre[:], 0.0)
                        relu_idx += 1
                        hpair.append(hT)

                    # ---- L2: 4 features col-tiled into one PSUM bank ----
                    pout = pout_pool.tile([128, CHUNK], dt.float32, tag="pout")
                    for j in range(4):
                        f = 4 * q + j
                        nc.tensor.matmul(
                            pout[32 * j:32 * j + 32, :],
                            lhsT=w2_sb[:, bass.ts(f, E)],
                            rhs=hpair[j // 2][:, bass.ts(j % 2, CHUNK)],
                            start=True, stop=True,
                            tile_position=(0, 32 * j),
                        )

                    # ---- + b2, PSUM -> SBUF bf16 ----
                    outq = outq_pool.tile([128, CHUNK], dt.bfloat16, tag="outq")
                    if copy_pat[copy_idx % len(copy_pat)] == "A":
                        nc.scalar.activation(
                            outq[:], pout[:],
                            mybir.ActivationFunctionType.Identity,
                            bias=b2_sb[:, q:q + 1],
                        )
                    else:
                        nc.vector.tensor_scalar_add(
                            outq[:], pout[:], b2_sb[:, q:q + 1])
                    copy_idx += 1

                    # rows fe = 128q + p, cols 512c + cc
                    nc.sync.dma_start(
                        out=out[bass.ts(q, 128), bass.ts(c, CHUNK)],
                        in_=outq[:],
                    )

    nc.compile()
    return nc


def _prep_inputs(x, w1, b1, w2, b2):
    """Host-side packing of parameters + per-core x shards."""
    w1b1q = np.zeros((128, F * H), dtype=BF16)
    for f in range(F):
        q, j = divmod(f, 4)
        w1b1q[32 * j + 0, H * q:H * q + H] = w1[f].astype(BF16)
        w1b1q[32 * j + 1, H * q:H * q + H] = b1[f].astype(BF16)

    w2s = np.ascontiguousarray(
        w2.transpose(1, 0, 2).reshape(H, F * E)).astype(BF16)
    # b2qs[32j + e, q] = b2[4q + j, e]
    b2qs = np.ascontiguousarray(
        b2.reshape(NQUAD, 4, E).transpose(1, 2, 0).reshape(128, NQUAD)
    ).astype(np.float32)

    in_maps = []
    for core in range(NCORES):
        xs = x[core * BL:(core + 1) * BL]          # [BL, F]
        xt2 = np.empty((2 * F, BL), dtype=BF16)
        xt2[0::2] = xs.T.astype(BF16)
        xt2[1::2] = BF16(1.0)
        in_maps.append({
            "xt2": xt2, "w1b1q": w1b1q, "w2s": w2s, "b2qs": b2qs,
        })
    return in_maps


def _get_compiled():
    global _COMPILED
    if _COMPILED is None:
        _COMPILED = _build_bass()
    return _COMPILED


def reset_compiled():
    global _COMPILED
    _COMPILED = None


def kernel(x, w1, b1, w2, b2, _trace=False, _trace_kwargs=None):
    nc = _get_compiled()
    in_maps = _prep_inputs(
        np.asarray(x, dtype=np.float32), np.asarray(w1, dtype=np.float32),
        np.asarray(b1, dtype=np.float32), np.asarray(w2, dtype=np.float32),
        np.asarray(b2, dtype=np.float32))
    res = run_bass_kernel_spmd(
        nc, in_maps, core_ids=list(range(NCORES)),
        trace=_trace, **(_trace_kwargs or {}))
    # outT [F*E, BL] bf16 per core -> [BL, F*E] fp32, concatenated over cores
    shards = [
        np.asarray(res.results[i]["out"]).astype(np.float32).T
        for i in range(NCORES)
    ]
    full = np.ascontiguousarray(np.concatenate(shards, axis=0))
    if _trace:
        return full, res
    return full


if __name__ == "__main__":
    rng = np.random.default_rng(0)
    x = rng.standard_normal((B, F), dtype=np.float32)
    w1 = rng.standard_normal((F, H), dtype=np.float32)
    b1 = rng.standard_normal((F, H), dtype=np.float32)
    w2 = (rng.standard_normal((F, H, E), dtype=np.float32) / np.sqrt(H)).astype(np.float32)
    b2 = rng.standard_normal((F, E), dtype=np.float32) / np.sqrt(H)
    got = kernel(x=x, w1=w1, b1=b1, w2=w2, b2=b2)
    h = np.maximum(x[:, :, None] * w1[None] + b1[None], 0.0)
    want = (np.einsum("bfh,fhe->bfe", h, w2) + b2[None]).reshape(B, F * E)
    err = np.abs(got - want).max() / np.abs(want).max()
    print("self-test scale-relative max err:", err)


# revision 3
# speedup vs baseline: 1.7529x; 1.0756x over previous
"""
Trainium2 Bass kernel for nn_DenseFeatureNumericEmbedding.

Computes, per feature f (F=128 independent tiny MLPs):
    h[b,f,:]   = relu(x[b,f] * w1[f,:] + b1[f,:])            # [B, F, H]
    out[b,f,:] = h[b,f,:] @ w2[f,:,:] + b2[f,:]              # [B, F, E]
    returns out.reshape(B, F*E)                              # [16384, 4096] fp32

Sharding: data-parallel over batch across 8 NeuronCores (2048 rows/core),
params replicated. No collectives; host concatenates the 8 output shards.

v3 dataflow: NO on-device transpose (kernel writes outT [F*E, BL] bf16;
host transposes + casts). Quad-OUTER loop: each quad of 4 features
processes all 4 batch chunks before moving on, so L1 stationaries are
reused across chunks and the PE matmul stream stays dense (HAM clock
gate un-throttles only under sustained PE activity; any sparse stream
runs at 1.2 GHz instead of 2.4 GHz).

Per quad (4 features = 2 pairs, row-groups 32j):
  xq   DMA per quad: 4x [2, 2048] descriptors (feature rows x/ones).
  L1   per pair, per chunk: 2 matmuls K=2 (bias folded) -> pre [128,1024]
       fp32 PSUM (2 banks); row-tiled pairs pack on the PE.
  RELU ScalarE activation(Relu) / VectorE tensor_scalar_max split,
       PSUM -> SBUF bf16 h tile [128, 1024].
  L2   per chunk: 4 matmuls col-tiled (M=32 each) into pout [128,512].
  COPY +b2 fused: ScalarE Identity+bias / VectorE tensor_scalar_add,
       PSUM -> SBUF bf16.
  DMA  outT tile [128, 512] -> DRAM rows 128q..128q+128.

PSUM: pre pool 3 bufs x 2 banks + pout 2 bufs x 1 bank = 8/8 banks.
"""

import sys

sys.path.insert(0, "/opt/trn_rl_repo")

import numpy as np
import ml_dtypes

import concourse.bass as bass
import concourse.tile as tile
from concourse import bacc, mybir
from concourse.bass_utils import run_bass_kernel_spmd

BF16 = ml_dtypes.bfloat16

B = 16384
F = 128
H = 128
E = 32
NCORES = 8
BL = B // NCORES          # 2048 rows per core
CHUNK = 512               # batch columns per inner tile (1 PSUM bank fp32)
NCHUNK = BL // CHUNK      # 4
NQUAD = F // 4            # 32 quads of 4 features

CONFIG = {
    # engine split patterns: 'A' = ScalarE (Act), 'D' = VectorE (DVE)
    "RELU_PAT": "AD",        # per relu instr (256 total), cycled
    "COPY_PAT": "AADADAAD",  # per copy instr (128 total), cycled
    "PRE_BUFS": 3,
    "POUT_BUFS": 2,
    "VARIANT_ID": 20,        # busts the NEFF cache between variants
}

_COMPILED = None


def _build_bass():
    nc = bacc.Bacc("TRN2", target_bir_lowering=False, debug=False,
                   num_devices=NCORES)
    dt = mybir.dt

    xt2 = nc.dram_tensor("xt2", [2 * F, BL], dt.bfloat16, kind="ExternalInput").ap()
    w1b1q = nc.dram_tensor("w1b1q", [128, F * H], dt.bfloat16, kind="ExternalInput").ap()
    w2s = nc.dram_tensor("w2s", [H, F * E], dt.bfloat16, kind="ExternalInput").ap()
    b2qs = nc.dram_tensor("b2qs", [128, NQUAD], dt.float32, kind="ExternalInput").ap()
    out = nc.dram_tensor("out", [F * E, BL], dt.bfloat16, kind="ExternalOutput").ap()

    # DRAM view of xt2: rows 8q + 2j + r (q quad, j feature-in-quad, r 0=x/1=ones)
    xt2_r = xt2.rearrange("(q g) n -> g q n", g=8)       # [8, NQUAD, BL]

    for _ in range(CONFIG["VARIANT_ID"]):
        nc.sync.nop()

    relu_pat = CONFIG["RELU_PAT"]
    copy_pat = CONFIG["COPY_PAT"]

    with tile.TileContext(nc) as tc:
        with (
            tc.tile_pool(name="params", bufs=1) as params,
            tc.tile_pool(name="xq", bufs=3) as xq_pool,
            tc.tile_pool(name="h", bufs=12) as h_pool,
            tc.tile_pool(name="outq", bufs=6) as outq_pool,
            tc.tile_pool(name="pre", bufs=CONFIG["PRE_BUFS"], space="PSUM") as pre_pool,
            tc.tile_pool(name="pout", bufs=CONFIG["POUT_BUFS"], space="PSUM") as pout_pool,
        ):
            w1b1q_sb = params.tile([128, F * H], dt.bfloat16, tag="w1b1q")
            nc.sync.dma_start(out=w1b1q_sb[:], in_=w1b1q[:])
            w2_sb = params.tile([H, F * E], dt.bfloat16, tag="w2s")
            nc.sync.dma_start(out=w2_sb[:], in_=w2s[:])
            b2_sb = params.tile([128, NQUAD], dt.float32, tag="b2qs")
            nc.sync.dma_start(out=b2_sb[:], in_=b2qs[:])

            relu_idx = 0
            copy_idx = 0
            for q in range(NQUAD):
                # xqt rows 32j+r = [x; ones] of feature 4q+j over full BL
                xqt = xq_pool.tile([128, BL], dt.bfloat16, tag="xq")
                for j in range(4):
                    nc.sync.dma_start(
                        out=xqt[32 * j:32 * j + 2, :],
                        in_=xt2_r[2 * j:2 * j + 2, q, :],
                    )

                hq = {}
                for p in range(2):          # pair p: features 4q+2p, 4q+2p+1
                    for c in range(NCHUNK):
                        pre = pre_pool.tile([128, 2 * CHUNK], dt.float32,
                                            tag="pre")
                        for jj in range(2):
                            j = 2 * p + jj
                            nc.tensor.matmul(
                                pre[:, bass.ts(jj, CHUNK)],
                                lhsT=w1b1q_sb[32 * j:32 * j + 2, bass.ts(q, H)],
                                rhs=xqt[32 * j:32 * j + 2, bass.ts(c, CHUNK)],
                                start=True, stop=True,
                                tile_position=(32 * j, 0),
                            )
                        hT = h_pool.tile([128, 2 * CHUNK], dt.bfloat16, tag="h")
                        if relu_pat[relu_idx % len(relu_pat)] == "A":
                            nc.scalar.activation(
                                hT[:], pre[:], mybir.ActivationFunctionType.Relu)
                        else:
                            nc.vector.tensor_scalar_max(hT[:], pre[:], 0.0)
                        relu_idx += 1
                        hq[(p, c)] = hT

                for c in range(NCHUNK):
                    # ---- L2: 4 features col-tiled into one PSUM bank ----
                    pout = pout_pool.tile([128, CHUNK], dt.float32, tag="pout")
                    for j in range(4):
                        f = 4 * q + j
                        nc.tensor.matmul(
                            pout[32 * j:32 * j + 32, :],
                            lhsT=w2_sb[:, bass.ts(f, E)],
                            rhs=hq[(j // 2, c)][:, bass.ts(j % 2, CHUNK)],
                            start=True, stop=True,
                            tile_position=(0, 32 * j),
                        )

                    # ---- + b2, PSUM -> SBUF bf16 ----
                    outq = outq_pool.tile([128, CHUNK], dt.bfloat16, tag="outq")
                    if copy_pat[copy_idx % len(copy_pat)] == "A":
                        nc.scalar.activation(
                            outq[:], pout[:],
                            mybir.ActivationFunctionType.Identity,
                            bias=b2_sb[:, q:q + 1],
                        )
                    else:
                        nc.vector.tensor_scalar_add(
                            outq[:], pout[:], b2_sb[:, q:q + 1])
                    copy_idx += 1

                    # rows fe = 128q + p, cols 512c + cc
                    nc.sync.dma_start(
                        out=out[bass.ts(q, 128), bass.ts(c, CHUNK)],
                        in_=outq[:],
                    )

    nc.compile()
    return nc


def _prep_inputs(x, w1, b1, w2, b2):
    """Host-side packing of parameters + per-core x shards."""
    w1b1q = np.zeros((128, F * H), dtype=BF16)
    for f in range(F):
        q, j = divmod(f, 4)
        w1b1q[32 * j + 0, H * q:H * q + H] = w1[f].astype(BF16)
        w1b1q[32 * j + 1, H * q:H * q + H] = b1[f].astype(BF16)

    w2s = np.ascontiguousarray(
        w2.transpose(1, 0, 2).reshape(H, F * E)).astype(BF16)
    # b2qs[32j + e, q] = b2[4q + j, e]
    b2qs = np.ascontiguousarray(
        b2.reshape(NQUAD, 4, E).transpose(1, 2, 0).reshape(128, NQUAD)
    ).astype(np.float32)

    in_maps = []
    for core in range(NCORES):
        xs = x[core * BL:(core + 1) * BL]          # [BL, F]
        xt2 = np.empty((2 * F, BL), dtype=BF16)
        xt2[0::2] = xs.T.astype(BF16)
        xt2[1::2] = BF16(1.0)
        in_maps.append({
            "xt2": xt2, "w1b1q": w1b1q, "w2s": w2s, "b2qs": b2qs,
        })
    return in_maps


def _get_compiled():
    global _COMPILED
    if _COMPILED is None:
        _COMPILED = _build_bass()
    return _COMPILED


def reset_compiled():
    global _COMPILED
    _COMPILED = None


def kernel(x, w1, b1, w2, b2, _trace=False, _trace_kwargs=None):
    nc = _get_compiled()
    in_maps = _prep_inputs(
        np.asarray(x, dtype=np.float32), np.asarray(w1, dtype=np.float32),
        np.asarray(b1, dtype=np.float32), np.asarray(w2, dtype=np.float32),
        np.asarray(b2, dtype=np.float32))
    res = run_bass_kernel_spmd(
        nc, in_maps, core_ids=list(range(NCORES)),
        trace=_trace, **(_trace_kwargs or {}))
    # outT [F*E, BL] bf16 per core -> [BL, F*E] fp32, concatenated over cores
    shards = [
        np.asarray(res.results[i]["out"]).astype(np.float32).T
        for i in range(NCORES)
    ]
    full = np.ascontiguousarray(np.concatenate(shards, axis=0))
    if _trace:
        return full, res
    return full


if __name__ == "__main__":
    rng = np.random.default_rng(0)
    x = rng.standard_normal((B, F), dtype=np.float32)
    w1 = rng.standard_normal((F, H), dtype=np.float32)
    b1 = rng.standard_normal((F, H), dtype=np.float32)
    w2 = (rng.standard_normal((F, H, E), dtype=np.float32) / np.sqrt(H)).astype(np.float32)
    b2 = rng.standard_normal((F, E), dtype=np.float32) / np.sqrt(H)
    got = kernel(x=x, w1=w1, b1=b1, w2=w2, b2=b2)
    h = np.maximum(x[:, :, None] * w1[None] + b1[None], 0.0)
    want = (np.einsum("bfh,fhe->bfe", h, w2) + b2[None]).reshape(B, F * E)
    err = np.abs(got - want).max() / np.abs(want).max()
    print("self-test scale-relative max err:", err)


# revision 5
# speedup vs baseline: 2.0125x; 1.1481x over previous
"""
Trainium2 Bass kernel for nn_DenseFeatureNumericEmbedding.

Computes, per feature f (F=128 independent tiny MLPs):
    h[b,f,:]   = relu(x[b,f] * w1[f,:] + b1[f,:])            # [B, F, H]
    out[b,f,:] = h[b,f,:] @ w2[f,:,:] + b2[f,:]              # [B, F, E]
    returns out.reshape(B, F*E)                              # [16384, 4096] fp32

Sharding: data-parallel over batch across 8 NeuronCores (2048 rows/core),
params replicated. No collectives; host concatenates the 8 output shards.

v5 dataflow:
 - NO on-device transpose: kernel writes outT [F*E, BL] bf16, host
   transposes + casts to fp32.
 - Quad-outer loop, all 4 batch chunks per quad: L1 stationaries are
   reused, PE matmul stream stays dense (HAM clock-gate friendly).
 - L1: per pair/chunk, 2 bf16 K=2 matmuls (bias folded via ones row),
   row-groups 32j -> pre [128,1024] fp32 PSUM; row-tiled pairs pack.
 - RELU ScalarE/VectorE split PSUM -> SBUF bf16.
 - L2: per chunk, 4 bf16 matmuls col-tiled (M=32) -> pout [128,512];
   software-pipelined: L2 of chunk c is emitted between later L1 chunks
   so the PE always has ready work while relu drains PSUM.
 - COPY +b2 fused (Identity+bias / tensor_scalar_add) -> outq [128,2048]
   staging; ONE output DMA per quad (4 KiB rows) on the Sync queue.
 - Input x DMAs ride the otherwise-idle GpSimd queue (descriptor
   generation for 128-row DMAs costs ~600ns each on the issuing queue).

PSUM: pre pool 3 bufs x 2 banks + pout 2 bufs x 1 bank = 8/8 banks.
"""

import sys

sys.path.insert(0, "/opt/trn_rl_repo")

import numpy as np
import ml_dtypes

import concourse.bass as bass
import concourse.tile as tile
from concourse import bacc, mybir
from concourse.bass_utils import run_bass_kernel_spmd

BF16 = ml_dtypes.bfloat16

B = 16384
F = 128
H = 128
E = 32
NCORES = 8
BL = B // NCORES          # 2048 rows per core
CHUNK = 512               # batch columns per inner tile (1 PSUM bank fp32)
NCHUNK = BL // CHUNK      # 4
NQUAD = F // 4            # 32 quads of 4 features

CONFIG = {
    "RELU_PAT": "ADADADADADADADADADAA",   # 11 A, 9 D per 20
    "COPY_PAT": "AD",
    "VARIANT_ID": 40,                     # busts the NEFF cache
}

_COMPILED = None


def _build_bass():
    nc = bacc.Bacc("TRN2", target_bir_lowering=False, debug=False,
                   num_devices=NCORES)
    dt = mybir.dt

    xt2 = nc.dram_tensor("xt2", [2 * F, BL], dt.bfloat16, kind="ExternalInput").ap()
    w1b1q = nc.dram_tensor("w1b1q", [128, F * H], dt.bfloat16, kind="ExternalInput").ap()
    w2s = nc.dram_tensor("w2s", [H, F * E], dt.bfloat16, kind="ExternalInput").ap()
    b2qs = nc.dram_tensor("b2qs", [128, NQUAD], dt.float32, kind="ExternalInput").ap()
    out = nc.dram_tensor("out", [F * E, BL], dt.bfloat16, kind="ExternalOutput").ap()

    # DRAM view of xt2: rows 8q + 2j + r (q quad, j feature-in-quad, r 0=x/1=ones)
    xt2_r = xt2.rearrange("(q g) n -> g q n", g=8)       # [8, NQUAD, BL]

    for _ in range(CONFIG["VARIANT_ID"]):
        nc.sync.nop()

    relu_pat = CONFIG["RELU_PAT"]
    copy_pat = CONFIG["COPY_PAT"]

    with tile.TileContext(nc) as tc:
        with (
            tc.tile_pool(name="params", bufs=1) as params,
            tc.tile_pool(name="xq", bufs=3) as xq_pool,
            tc.tile_pool(name="h", bufs=10) as h_pool,
            tc.tile_pool(name="outq", bufs=3) as outq_pool,
            tc.tile_pool(name="pre", bufs=3, space="PSUM") as pre_pool,
            tc.tile_pool(name="pout", bufs=2, space="PSUM") as pout_pool,
        ):
            w1b1q_sb = params.tile([128, F * H], dt.bfloat16, tag="w1b1q")
            nc.sync.dma_start(out=w1b1q_sb[:], in_=w1b1q[:])
            w2_sb = params.tile([H, F * E], dt.bfloat16, tag="w2s")
            nc.sync.dma_start(out=w2_sb[:], in_=w2s[:])
            b2_sb = params.tile([128, NQUAD], dt.float32, tag="b2qs")
            nc.sync.dma_start(out=b2_sb[:], in_=b2qs[:])

            relu_idx = 0
            copy_idx = 0
            for q in range(NQUAD):
                # xqt rows 32j+r = [x; ones] of feature 4q+j over full BL
                xqt = xq_pool.tile([128, BL], dt.bfloat16, tag="xq")
                for j in range(4):
                    nc.gpsimd.dma_start(
                        out=xqt[32 * j:32 * j + 2, :],
                        in_=xt2_r[2 * j:2 * j + 2, q, :],
                    )

                hq = {}
                outq = outq_pool.tile([128, NCHUNK * CHUNK], dt.bfloat16,
                                      tag="outq")

                def do_l1(c):
                    nonlocal relu_idx
                    for p in range(2):      # pair p: features 4q+2p, 4q+2p+1
                        pre = pre_pool.tile([128, 2 * CHUNK], dt.float32,
                                            tag="pre")
                        for jj in range(2):
                            j = 2 * p + jj
                            nc.tensor.matmul(
                                pre[:, bass.ts(jj, CHUNK)],
                                lhsT=w1b1q_sb[32 * j:32 * j + 2,
                                              bass.ts(q, H)],
                                rhs=xqt[32 * j:32 * j + 2,
                                        bass.ts(c, CHUNK)],
                                start=True, stop=True,
                                tile_position=(32 * j, 0),
                            )
                        hT = h_pool.tile([128, 2 * CHUNK], dt.bfloat16,
                                         tag="h")
                        if relu_pat[relu_idx % len(relu_pat)] == "A":
                            nc.scalar.activation(
                                hT[:], pre[:],
                                mybir.ActivationFunctionType.Relu)
                        else:
                            nc.vector.tensor_scalar_max(hT[:], pre[:], 0.0)
                        relu_idx += 1
                        hq[(p, c)] = hT

                def do_l2(c):
                    nonlocal copy_idx
                    pout = pout_pool.tile([128, CHUNK], dt.float32,
                                          tag="pout")
                    for j in range(4):
                        f = 4 * q + j
                        nc.tensor.matmul(
                            pout[32 * j:32 * j + 32, :],
                            lhsT=w2_sb[:, bass.ts(f, E)],
                            rhs=hq[(j // 2, c)][:, bass.ts(j % 2, CHUNK)],
                            start=True, stop=True,
                            tile_position=(0, 32 * j),
                        )
                    dst = outq[:, bass.ts(c, CHUNK)]
                    if copy_pat[copy_idx % len(copy_pat)] == "A":
                        nc.scalar.activation(
                            dst, pout[:],
                            mybir.ActivationFunctionType.Identity,
                            bias=b2_sb[:, q:q + 1],
                        )
                    else:
                        nc.vector.tensor_scalar_add(
                            dst, pout[:], b2_sb[:, q:q + 1])
                    copy_idx += 1

                # software pipeline: L2(c) interleaved between later L1 chunks
                do_l1(0)
                do_l1(1)
                do_l2(0)
                do_l1(2)
                do_l2(1)
                do_l1(3)
                do_l2(2)
                do_l2(3)

                # one output DMA per quad: rows 128q..128q+128, all BL cols
                nc.sync.dma_start(out=out[bass.ts(q, 128), :], in_=outq[:])

    nc.compile()
    return nc


def _prep_inputs(x, w1, b1, w2, b2):
    """Host-side packing of parameters + per-core x shards."""
    w1b1q = np.zeros((128, F * H), dtype=BF16)
    for f in range(F):
        q, j = divmod(f, 4)
        w1b1q[32 * j + 0, H * q:H * q + H] = w1[f].astype(BF16)
        w1b1q[32 * j + 1, H * q:H * q + H] = b1[f].astype(BF16)

    w2s = np.ascontiguousarray(
        w2.transpose(1, 0, 2).reshape(H, F * E)).astype(BF16)
    # b2qs[32j + e, q] = b2[4q + j, e]
    b2qs = np.ascontiguousarray(
        b2.reshape(NQUAD, 4, E).transpose(1, 2, 0).reshape(128, NQUAD)
    ).astype(np.float32)

    in_maps = []
    for core in range(NCORES):
        xs = x[core * BL:(core + 1) * BL]          # [BL, F]
        xt2 = np.empty((2 * F, BL), dtype=BF16)
        xt2[0::2] = xs.T.astype(BF16)
        xt2[1::2] = BF16(1.0)
        in_maps.append({
            "xt2": xt2, "w1b1q": w1b1q, "w2s": w2s, "b2qs": b2qs,
        })
    return in_maps


def _get_compiled():
    global _COMPILED
    if _COMPILED is None:
        _COMPILED = _build_bass()
    return _COMPILED


def reset_compiled():
    global _COMPILED
    _COMPILED = None


def kernel(x, w1, b1, w2, b2, _trace=False, _trace_kwargs=None):
    nc = _get_compiled()
    in_maps = _prep_inputs(
        np.asarray(x, dtype=np.float32), np.asarray(w1, dtype=np.float32),
        np.asarray(b1, dtype=np.float32), np.asarray(w2, dtype=np.float32),
        np.asarray(b2, dtype=np.float32))
    res = run_bass_kernel_spmd(
        nc, in_maps, core_ids=list(range(NCORES)),
        trace=_trace, **(_trace_kwargs or {}))
    # outT [F*E, BL] bf16 per core -> [BL, F*E] fp32, concatenated over cores
    shards = [
        np.asarray(res.results[i]["out"]).astype(np.float32).T
        for i in range(NCORES)
    ]
    full = np.ascontiguousarray(np.concatenate(shards, axis=0))
    if _trace:
        return full, res
    return full


if __name__ == "__main__":
    rng = np.random.default_rng(0)
    x = rng.standard_normal((B, F), dtype=np.float32)
    w1 = rng.standard_normal((F, H), dtype=np.float32)
    b1 = rng.standard_normal((F, H), dtype=np.float32)
    w2 = (rng.standard_normal((F, H, E), dtype=np.float32) / np.sqrt(H)).astype(np.float32)
    b2 = rng.standard_normal((F, E), dtype=np.float32) / np.sqrt(H)
    got = kernel(x=x, w1=w1, b1=b1, w2=w2, b2=b2)
    h = np.maximum(x[:, :, None] * w1[None] + b1[None], 0.0)
    want = (np.einsum("bfh,fhe->bfe", h, w2) + b2[None]).reshape(B, F * E)
    err = np.abs(got - want).max() / np.abs(want).max()
    print("self-test scale-relative max err:", err)


# revision 8
# speedup vs baseline: 2.2761x; 1.1310x over previous
"""
Trainium2 Bass kernel for nn_DenseFeatureNumericEmbedding.

Computes, per feature f (F=128 independent tiny MLPs):
    h[b,f,:]   = relu(x[b,f] * w1[f,:] + b1[f,:])            # [B, F, H]
    out[b,f,:] = h[b,f,:] @ w2[f,:,:] + b2[f,:]              # [B, F, E]
    returns out.reshape(B, F*E)                              # [16384, 4096] fp32

Sharding: data-parallel over batch across 8 NeuronCores (2048 rows/core),
params replicated. No collectives; host concatenates the 8 output shards.

v5 dataflow:
 - NO on-device transpose: kernel writes outT [F*E, BL] bf16, host
   transposes + casts to fp32.
 - Quad-outer loop, all 4 batch chunks per quad: L1 stationaries are
   reused, PE matmul stream stays dense (HAM clock-gate friendly).
 - L1: per pair/chunk, 2 bf16 K=2 matmuls (bias folded via ones row),
   row-groups 32j -> pre [128,1024] fp32 PSUM; row-tiled pairs pack.
 - RELU ScalarE/VectorE split PSUM -> SBUF bf16.
 - L2: per chunk, 4 bf16 matmuls col-tiled (M=32) -> pout [128,512];
   software-pipelined: L2 of chunk c is emitted between later L1 chunks
   so the PE always has ready work while relu drains PSUM.
 - COPY +b2 fused (Identity+bias / tensor_scalar_add) -> outq [128,2048]
   staging; ONE output DMA per quad (4 KiB rows) on the Sync queue.
 - Input x DMAs ride the otherwise-idle GpSimd queue (descriptor
   generation for 128-row DMAs costs ~600ns each on the issuing queue).

PSUM: pre pool 3 bufs x 2 banks + pout 2 bufs x 1 bank = 8/8 banks.
"""

import sys

sys.path.insert(0, "/opt/trn_rl_repo")

import numpy as np
import ml_dtypes

import concourse.bass as bass
import concourse.tile as tile
from concourse import bacc, mybir
from concourse.bass_utils import run_bass_kernel_spmd

BF16 = ml_dtypes.bfloat16

B = 16384
F = 128
H = 128
E = 32
NCORES = 8
BL = B // NCORES          # 2048 rows per core
CHUNK = 512               # batch columns per inner tile (1 PSUM bank fp32)
NCHUNK = BL // CHUNK      # 4
NQUAD = F // 4            # 32 quads of 4 features

CONFIG = {
    "RELU_PAT": "ADADADADADADADADADADADADA",   # 13 A, 12 D per 25
    "COPY_PAT": "AD",
    "VARIANT_ID": 50,                          # busts the NEFF cache
}

_COMPILED = None


def _build_bass():
    nc = bacc.Bacc("TRN2", target_bir_lowering=False, debug=False,
                   num_devices=NCORES)
    dt = mybir.dt

    xt2 = nc.dram_tensor("xt2", [2 * F, BL], dt.bfloat16, kind="ExternalInput").ap()
    w1b1q = nc.dram_tensor("w1b1q", [128, F * H], dt.bfloat16, kind="ExternalInput").ap()
    w2s = nc.dram_tensor("w2s", [H, F * E], dt.bfloat16, kind="ExternalInput").ap()
    b2qs = nc.dram_tensor("b2qs", [128, NQUAD], dt.float32, kind="ExternalInput").ap()
    out = nc.dram_tensor("out", [F * E, BL], dt.bfloat16, kind="ExternalOutput").ap()

    # DRAM view of xt2: rows 8q + 2j + r (q quad, j feature-in-quad, r 0=x/1=ones)
    xt2_r = xt2.rearrange("(q g) n -> g q n", g=8)       # [8, NQUAD, BL]

    for _ in range(CONFIG["VARIANT_ID"]):
        nc.sync.nop()

    relu_pat = CONFIG["RELU_PAT"]
    copy_pat = CONFIG["COPY_PAT"]

    with tile.TileContext(nc) as tc:
        with (
            tc.tile_pool(name="params", bufs=1) as params,
            tc.tile_pool(name="xq", bufs=3) as xq_pool,
            tc.tile_pool(name="h", bufs=10) as h_pool,
            tc.tile_pool(name="outq", bufs=3) as outq_pool,
            tc.tile_pool(name="pre", bufs=3, space="PSUM") as pre_pool,
            tc.tile_pool(name="pout", bufs=2, space="PSUM") as pout_pool,
        ):
            # params split into quad-group pieces so quad 0's matmuls can
            # start after ~1/8 of the parameter traffic has landed
            w1b1q_sb = params.tile([128, F * H], dt.bfloat16, tag="w1b1q")
            w2_sb = params.tile([H, F * E], dt.bfloat16, tag="w2s")
            b2_sb = params.tile([128, NQUAD], dt.float32, tag="b2qs")
            nc.sync.dma_start(out=b2_sb[:], in_=b2qs[:])
            NSPLIT = 8
            for s in range(NSPLIT):
                nc.sync.dma_start(
                    out=w1b1q_sb[:, bass.ts(s, F * H // NSPLIT)],
                    in_=w1b1q[:, bass.ts(s, F * H // NSPLIT)])
                nc.sync.dma_start(
                    out=w2_sb[:, bass.ts(s, F * E // NSPLIT)],
                    in_=w2s[:, bass.ts(s, F * E // NSPLIT)])

            relu_idx = 0
            copy_idx = 0

            def make_quad(q):
                # xqt rows 32j+r = [x; ones] of feature 4q+j over full BL
                xqt = xq_pool.tile([128, BL], dt.bfloat16, tag="xq")
                for j in range(4):
                    nc.gpsimd.dma_start(
                        out=xqt[32 * j:32 * j + 2, :],
                        in_=xt2_r[2 * j:2 * j + 2, q, :],
                    )
                hq = {}
                outq = outq_pool.tile([128, NCHUNK * CHUNK], dt.bfloat16,
                                      tag="outq")

                def do_l1(c):
                    nonlocal relu_idx
                    for p in range(2):      # pair p: features 4q+2p, 4q+2p+1
                        pre = pre_pool.tile([128, 2 * CHUNK], dt.float32,
                                            tag="pre")
                        for jj in range(2):
                            j = 2 * p + jj
                            nc.tensor.matmul(
                                pre[:, bass.ts(jj, CHUNK)],
                                lhsT=w1b1q_sb[32 * j:32 * j + 2,
                                              bass.ts(q, H)],
                                rhs=xqt[32 * j:32 * j + 2,
                                        bass.ts(c, CHUNK)],
                                start=True, stop=True,
                                tile_position=(32 * j, 0),
                            )
                        hT = h_pool.tile([128, 2 * CHUNK], dt.bfloat16,
                                         tag="h")
                        if relu_pat[relu_idx % len(relu_pat)] == "A":
                            nc.scalar.activation(
                                hT[:], pre[:],
                                mybir.ActivationFunctionType.Relu)
                        else:
                            nc.vector.tensor_scalar_max(hT[:], pre[:], 0.0)
                        relu_idx += 1
                        hq[(p, c)] = hT

                def do_l2(c):
                    nonlocal copy_idx
                    pout = pout_pool.tile([128, CHUNK], dt.float32,
                                          tag="pout")
                    for j in range(4):
                        f = 4 * q + j
                        nc.tensor.matmul(
                            pout[32 * j:32 * j + 32, :],
                            lhsT=w2_sb[:, bass.ts(f, E)],
                            rhs=hq[(j // 2, c)][:, bass.ts(j % 2, CHUNK)],
                            start=True, stop=True,
                            tile_position=(0, 32 * j),
                        )
                    dst = outq[:, bass.ts(c, CHUNK)]
                    if copy_pat[copy_idx % len(copy_pat)] == "A":
                        nc.scalar.activation(
                            dst, pout[:],
                            mybir.ActivationFunctionType.Identity,
                            bias=b2_sb[:, q:q + 1],
                        )
                    else:
                        nc.vector.tensor_scalar_add(
                            dst, pout[:], b2_sb[:, q:q + 1])
                    copy_idx += 1

                def do_dma():
                    nc.sync.dma_start(out=out[bass.ts(q, 128), :],
                                      in_=outq[:])

                return do_l1, do_l2, do_dma

            # software pipeline across quads: the last L2 + output DMA of
            # quad q are deferred until after quad q+1's first L1 chunk, so
            # the PE always has L1 work ready behind the K=128 L2 matmuls
            # (whose row-group footprint blocks LDWEIGHTS pull-ahead).
            pending = None
            for q in range(NQUAD):
                do_l1, do_l2, do_dma = make_quad(q)
                do_l1(0)
                if pending is not None:
                    pl2, pdma = pending
                    pl2(3)
                    pdma()
                do_l1(1)
                do_l2(0)
                do_l1(2)
                do_l2(1)
                do_l1(3)
                do_l2(2)
                pending = (do_l2, do_dma)
            pl2, pdma = pending
            pl2(3)
            pdma()

    nc.compile()
    return nc


def _prep_inputs(x, w1, b1, w2, b2):
    """Host-side packing of parameters + per-core x shards."""
    w1b1q = np.zeros((128, F * H), dtype=BF16)
    for f in range(F):
        q, j = divmod(f, 4)
        w1b1q[32 * j + 0, H * q:H * q + H] = w1[f].astype(BF16)
        w1b1q[32 * j + 1, H * q:H * q + H] = b1[f].astype(BF16)

    w2s = np.ascontiguousarray(
        w2.transpose(1, 0, 2).reshape(H, F * E)).astype(BF16)
    # b2qs[32j + e, q] = b2[4q + j, e]
    b2qs = np.ascontiguousarray(
        b2.reshape(NQUAD, 4, E).transpose(1, 2, 0).reshape(128, NQUAD)
    ).astype(np.float32)

    in_maps = []
    for core in range(NCORES):
        xs = x[core * BL:(core + 1) * BL]          # [BL, F]
        xt2 = np.empty((2 * F, BL), dtype=BF16)
        xt2[0::2] = xs.T.astype(BF16)
        xt2[1::2] = BF16(1.0)
        in_maps.append({
            "xt2": xt2, "w1b1q": w1b1q, "w2s": w2s, "b2qs": b2qs,
        })
    return in_maps


def _get_compiled():
    global _COMPILED
    if _COMPILED is None:
        _COMPILED = _build_bass()
    return _COMPILED


def reset_compiled():
    global _COMPILED
    _COMPILED = None


def kernel(x, w1, b1, w2, b2, _trace=False, _trace_kwargs=None):
    nc = _get_compiled()
    in_maps = _prep_inputs(
        np.asarray(x, dtype=np.float32), np.asarray(w1, dtype=np.float32),
        np.asarray(b1, dtype=np.float32), np.asarray(w2, dtype=np.float32),
        np.asarray(b2, dtype=np.float32))
    res = run_bass_kernel_spmd(
        nc, in_maps, core_ids=list(range(NCORES)),
        trace=_trace, **(_trace_kwargs or {}))
    # outT [F*E, BL] bf16 per core -> [BL, F*E] fp32, concatenated over cores
    shards = [
        np.asarray(res.results[i]["out"]).astype(np.float32).T
        for i in range(NCORES)
    ]
    full = np.ascontiguousarray(np.concatenate(shards, axis=0))
    if _trace:
        return full, res
    return full


if __name__ == "__main__":
    rng = np.random.default_rng(0)
    x = rng.standard_normal((B, F), dtype=np.float32)
    w1 = rng.standard_normal((F, H), dtype=np.float32)
    b1 = rng.standard_normal((F, H), dtype=np.float32)
    w2 = (rng.standard_normal((F, H, E), dtype=np.float32) / np.sqrt(H)).astype(np.float32)
    b2 = rng.standard_normal((F, E), dtype=np.float32) / np.sqrt(H)
    got = kernel(x=x, w1=w1, b1=b1, w2=w2, b2=b2)
    h = np.maximum(x[:, :, None] * w1[None] + b1[None], 0.0)
    want = (np.einsum("bfh,fhe->bfe", h, w2) + b2[None]).reshape(B, F * E)
    err = np.abs(got - want).max() / np.abs(want).max()
    print("self-test scale-relative max err:", err)
